# revision 9
# baseline (speedup 1.0000x reference)
"""Trainium2 Bass kernel for the ALS digital twin (2-layer GRU-D + heads).

Self-contained: hardcodes all shapes. Data-parallel over 8 NeuronCores
(batch 256 -> 32 per core). All tensors are kept feature-major
([feature, batch] / [feature, time*batch]) so every matmul contracts over
the partition dimension with zero on-device transposes; all layout
transforms happen host-side in numpy.

Per core the sequence is processed in 64 sub-chunks of T=8 steps:
  1. gamma tensors for the sub-chunk (outer-product matmuls + Exp)
  2. x_imp = x * max(mask, gamma_x)   (exact: mask is 0/1, gamma_x <= 1)
  3. U0 = W0x @ [x_imp; 1] written straight into PSUM (bias via ones row)
  4. layer-0 scan: 8 steps; recurrent gate matmuls ACCUMULATE onto the
     U-values already sitting in PSUM (start=False), activations read
     PSUM directly -> no adds/copies for input contributions.
  5. U1 = W1x @ H0 + b1 into PSUM, then layer-1 scan the same way,
     plus a one-hot (t == seq_len-1) accumulate of the final hidden.
Steps alternate PSUM banks so the activation read of step t overlaps the
matmuls of step t+1. Weights/state are bf16 (PE fast-weight-load),
accumulation is fp32 in PSUM. sigmoid(x) = 0.5*tanh(x/2)+0.5 keeps the
whole kernel in the single exp/tanh/relu ACT table set.
"""

import numpy as np
import ml_dtypes

import concourse.bass as bass
import concourse.bacc as bacc
import concourse.mybir as mybir
import concourse.tile as tile
from concourse.bass_utils import run_bass_kernel_spmd

BF16 = mybir.dt.bfloat16
F32 = mybir.dt.float32
NPBF = ml_dtypes.bfloat16

B, S, DT, DS, H = 256, 512, 64, 64, 256
NCORES = 8
BC = B // NCORES          # 32 batch per core
T = 8                     # steps per sub-chunk
NSC = S // T              # 64 sub-chunks
NUM_EVENTS, NUM_INTERVALS, NUM_STATE = 5, 20, 6
HEAD_H = 64
OUT_D = NUM_STATE + NUM_EVENTS * NUM_INTERVALS  # 106

AF = mybir.ActivationFunctionType
ALU = mybir.AluOpType


# ----------------------------------------------------------------------------
# device kernel builder
# ----------------------------------------------------------------------------

def build_nc():
    nc = bacc.Bacc(
        "TRN2", target_bir_lowering=False, debug=False, num_devices=NCORES)

    # --- DRAM parameters -----------------------------------------------------
    def par(name, shape, dtype=BF16, out=False):
        return nc.declare_dram_parameter(name, list(shape), dtype, isOutput=out)

    d_temporal = par("temporal_t", (DT, S, BC))
    d_mask = par("mask_t", (DT, S, BC))
    d_delta = par("delta_t", (1, S * BC))
    d_lm1 = par("lm1", (128, 64), F32)
    d_static = par("static_aug", (DS + 1, BC))

    d_w0x = par("w0x", (DT + 1, 768))
    d_w0h = par("w0h", (128, 1536))
    d_w1x = par("w1x", (128, 1536))
    d_w1h = par("w1h", (128, 1536))
    d_b1 = par("b1", (1, 768))
    d_wgh0 = par("wgh0", (1, 256))
    d_wgx0 = par("wgx0", (1, 64))
    d_wgh1 = par("wgh1", (1, 256))
    d_bgh0n = par("bgh0n", (128, 2), F32)
    d_bgx0n = par("bgx0n", (64, 1), F32)
    d_bgh1n = par("bgh1n", (128, 2), F32)

    d_ws1 = par("ws1", (DS + 1, 256))
    d_ws2 = par("ws2", (128, 512))
    d_bs2 = par("bs2", (1, 256))
    d_wfu = par("wfu", (128, 1024))
    d_bfu = par("bfu", (1, 256))
    d_wh0 = par("wh0", (128, 768))
    d_bh0 = par("bh0", (1, 384))
    d_wh1 = par("wh1", (64, 384))
    d_bh1 = par("bh1", (1, 384))
    d_wh2 = par("wh2", (64, OUT_D))
    d_bh2 = par("bh2", (1, OUT_D))

    d_out = par("out", (OUT_D, BC), F32, out=True)

    with tile.TileContext(nc) as tc:
        with (
            tc.tile_pool(name="cpool", bufs=1) as cpool,
            tc.tile_pool(name="spool", bufs=2) as spool,
            tc.tile_pool(name="scp", bufs=3) as scp,
        ):
            # --- constants into SBUF ----------------------------------------
            def load(pool, dram, shape, dtype=BF16, tag=None):
                t_ = pool.tile(list(shape), dtype, tag=tag or dram.name)
                nc.sync.dma_start(out=t_[:], in_=dram[:])
                return t_

            w0x = load(cpool, d_w0x, (DT + 1, 768))
            w0h = load(cpool, d_w0h, (128, 1536))
            w1x = load(cpool, d_w1x, (128, 1536))
            w1h = load(cpool, d_w1h, (128, 1536))
            b1t = load(cpool, d_b1, (1, 768))
            wgh0 = load(cpool, d_wgh0, (1, 256))
            wgx0 = load(cpool, d_wgx0, (1, 64))
            wgh1 = load(cpool, d_wgh1, (1, 256))
            bgh0n = load(cpool, d_bgh0n, (128, 2), F32)
            bgx0n = load(cpool, d_bgx0n, (64, 1), F32)
            bgh1n = load(cpool, d_bgh1n, (128, 2), F32)
            lm1 = load(cpool, d_lm1, (128, 64), F32)
            d_all = load(cpool, d_delta, (1, S * BC))

            ones128 = cpool.tile([1, 128], BF16, tag="ones128")
            nc.vector.memset(ones128[:], 1.0)
            zeros_h = cpool.tile([128, 64], BF16, tag="zeros_h")
            nc.vector.memset(zeros_h[:], 0.0)
            acc = cpool.tile([128, 64], BF16, tag="acc")
            nc.vector.memset(acc[:], 0.0)

            h0_prev = zeros_h[:, :]
            h1_prev = zeros_h[:, :]

            with (
                tc.tile_pool(name="psA", bufs=2, space="PSUM") as psA,
                tc.tile_pool(name="psB", bufs=1, space="PSUM") as psB,
                tc.tile_pool(name="psG", bufs=2, space="PSUM") as psG,
            ):
                def gamma(wg, bgn, m_chunks, d_sl, out_tile):
                    """out[p, t*64 + m*32 + b] = exp(-(w[p] d + b[p])), <=1."""
                    npart = 64 if m_chunks == 1 else 128
                    pg = psG.tile([128, 512], F32, tag="pg")
                    for m in range(m_chunks):
                        nc.tensor.matmul(
                            pg[:npart, m * 256:(m + 1) * 256],
                            wg[:, m * 128:m * 128 + npart],
                            d_sl,
                            start=True, stop=True,
                        )
                    for m in range(m_chunks):
                        if m_chunks == 1:
                            o = out_tile[:, :].rearrange(
                                "p (t b) -> p t b", b=BC)
                        else:
                            o = out_tile[:, :].rearrange(
                                "p (t c b) -> p t c b", c=2, b=BC)[:, :, m, :]
                        nc.scalar.activation(
                            o, pg[:npart, m * 256:(m + 1) * 256].rearrange(
                                "p (t b) -> p t b", b=BC),
                            AF.Exp, bias=bgn[:npart, m:m + 1], scale=-1.0)
                    nc.vector.tensor_scalar_min(out_tile[:], out_tile[:], 1.0)

                def grud_scan(t_loc, h_prev_sl, wh, gh, hbuf, pz, ph):
                    """One GRU-D step; U-values already accumulated in psum."""
                    bz = (t_loc % 2) * 512 + (t_loc // 2) * 32
                    hdec = scp.tile([128, 64], BF16, tag="hdec")
                    nc.vector.tensor_mul(
                        hdec[:], h_prev_sl, gh[:, t_loc * 64:(t_loc + 1) * 64])
                    for m in range(4):
                        for k in range(2):
                            nc.tensor.matmul(
                                pz[:, bz + m * 128: bz + m * 128 + 32],
                                wh[:, (k * 6 + m) * 128:(k * 6 + m + 1) * 128],
                                hdec[:, k * 32:(k + 1) * 32],
                                start=False, stop=(k == 1),
                                skip_group_check=True,
                            )
                    zr = scp.tile([128, 128], BF16, tag="zr")
                    pz_v = pz[:, :].rearrange(
                        "p (c m t b) -> p c m t b", c=2, m=4, t=T // 2, b=BC)
                    nc.scalar.activation(
                        zr[:, :].rearrange("p (m b) -> p m b", b=BC),
                        pz_v[:, t_loc % 2, :, t_loc // 2, :],
                        AF.Tanh, scale=0.5)
                    nc.vector.tensor_scalar(
                        zr[:], zr[:], 0.5, 0.5, ALU.mult, ALU.add)
                    rh = scp.tile([128, 64], BF16, tag="rh")
                    nc.vector.tensor_mul(rh[:], zr[:, 64:128], hdec[:])
                    for m in range(2):
                        for k in range(2):
                            nc.tensor.matmul(
                                ph[:, bz + m * 128: bz + m * 128 + 32],
                                wh[:, (k * 6 + 4 + m) * 128:(k * 6 + 5 + m) * 128],
                                rh[:, k * 32:(k + 1) * 32],
                                start=False, stop=(k == 1),
                                skip_group_check=True,
                            )
                    ht = scp.tile([128, 64], BF16, tag="ht")
                    ph_v = ph[:, :].rearrange(
                        "p (c m t b) -> p c m t b", c=2, m=4, t=T // 2, b=BC)
                    nc.scalar.activation(
                        ht[:, :].rearrange("p (m b) -> p m b", b=BC),
                        ph_v[:, t_loc % 2, 0:2, t_loc // 2, :],
                        AF.Tanh)
                    dl = scp.tile([128, 64], BF16, tag="dl")
                    nc.vector.tensor_sub(dl[:], ht[:], hdec[:])
                    nc.vector.tensor_mul(dl[:], zr[:, 0:64], dl[:])
                    out_sl = hbuf[:, t_loc * 64:(t_loc + 1) * 64]
                    nc.vector.tensor_add(out_sl, hdec[:], dl[:])
                    return out_sl

                for sc_i in range(NSC):
                    t0 = sc_i * T
                    d_sl = d_all[:, t0 * BC:(t0 + T) * BC]

                    # gammas
                    gh0 = spool.tile([128, 512], BF16, tag="gh0")
                    gamma(wgh0, bgh0n, 2, d_sl, gh0)
                    gx0 = spool.tile([64, 256], BF16, tag="gx0")
                    gamma(wgx0, bgx0n, 1, d_sl, gx0)
                    gh1 = spool.tile([128, 512], BF16, tag="gh1")
                    gamma(wgh1, bgh1n, 2, d_sl, gh1)

                    # x_imp
                    xt = spool.tile([DT, T * BC], BF16, tag="xt")
                    nc.sync.dma_start(
                        out=xt[:, :].rearrange("p (t b) -> p t b", b=BC),
                        in_=d_temporal[:, t0:t0 + T, :])
                    mt = spool.tile([DT, T * BC], BF16, tag="mt")
                    nc.sync.dma_start(
                        out=mt[:, :].rearrange("p (t b) -> p t b", b=BC),
                        in_=d_mask[:, t0:t0 + T, :])
                    ximp = spool.tile([DT + 1, T * BC], BF16, tag="ximp")
                    nc.vector.tensor_max(ximp[0:DT, :], mt[:], gx0[:])
                    nc.vector.tensor_mul(ximp[0:DT, :], ximp[0:DT, :], xt[:])
                    nc.vector.memset(ximp[DT:DT + 1, :], 1.0)

                    # U0 into psum (bias via ones row of ximp)
                    pz0 = psA.tile([128, 1024], F32, tag="pz")
                    ph0 = psB.tile([128, 1024], F32, tag="ph")
                    xv = ximp[:, :].rearrange(
                        "p (t2 c b) -> p t2 c b", c=2, b=BC)
                    for m in range(6):
                        dst = pz0 if m < 4 else ph0
                        mm = m if m < 4 else m - 4
                        for c in range(2):
                            nc.tensor.matmul(
                                dst[:, c * 512 + mm * 128:
                                    c * 512 + (mm + 1) * 128],
                                w0x[:, m * 128:(m + 1) * 128],
                                xv[:, :, c, :],
                                start=True, stop=True,
                            )

                    # layer-0 scan
                    h0buf = spool.tile([128, T * 64], BF16, tag="h0buf")
                    for t in range(T):
                        h0_prev = grud_scan(t, h0_prev, w0h, gh0, h0buf,
                                            pz0, ph0)

                    # U1 into psum
                    pz1 = psA.tile([128, 1024], F32, tag="pz")
                    ph1 = psB.tile([128, 1024], F32, tag="ph")
                    hv = h0buf[:, :].rearrange(
                        "p (t2 c x) -> p t2 c x", c=2, x=64)
                    for m in range(6):
                        dst = pz1 if m < 4 else ph1
                        mm = m if m < 4 else m - 4
                        for c in range(2):
                            for k in range(2):
                                nc.tensor.matmul(
                                    dst[:, c * 512 + mm * 128:
                                        c * 512 + (mm + 1) * 128],
                                    w1x[:, (k * 6 + m) * 128:
                                        (k * 6 + m + 1) * 128],
                                    hv[:, :, c, k * 32:(k + 1) * 32],
                                    start=(k == 0), stop=(k == 1),
                                )
                            nc.tensor.matmul(
                                dst[:, c * 512 + mm * 128:
                                    c * 512 + (mm + 1) * 128],
                                b1t[:, m * 128:(m + 1) * 128],
                                ones128[:],
                                start=False, stop=True,
                                skip_group_check=True,
                            )

                    # layer-1 scan + final-hidden select
                    h1buf = spool.tile([128, T * 64], BF16, tag="h1buf")
                    for t in range(T):
                        h1_prev = grud_scan(t, h1_prev, w1h, gh1, h1buf,
                                            pz1, ph1)
                        selt = scp.tile([128, 64], BF16, tag="selt")
                        nc.vector.tensor_scalar(
                            selt[:], lm1[:], float(t0 + t), None, ALU.is_equal)
                        nc.vector.tensor_mul(selt[:], selt[:], h1_prev)
                        nc.vector.tensor_add(acc[:], acc[:], selt[:])

            # --- heads ------------------------------------------------------
            with tc.tile_pool(name="psH", bufs=1, space="PSUM") as psH:
                statict = load(cpool, d_static, (DS + 1, BC))
                ws1 = load(cpool, d_ws1, (DS + 1, 256))
                ws2 = load(cpool, d_ws2, (128, 512))
                bs2 = load(cpool, d_bs2, (1, 256))
                wfu = load(cpool, d_wfu, (128, 1024))
                bfu = load(cpool, d_bfu, (1, 256))
                wh0 = load(cpool, d_wh0, (128, 768))
                bh0 = load(cpool, d_bh0, (1, 384))
                wh1 = load(cpool, d_wh1, (64, 384))
                bh1 = load(cpool, d_bh1, (1, 384))
                wh2 = load(cpool, d_wh2, (64, OUT_D))
                bh2 = load(cpool, d_bh2, (1, OUT_D))
                ones32 = cpool.tile([1, BC], BF16, tag="ones32")
                nc.vector.memset(ones32[:], 1.0)

                # static MLP: s1
                ps1 = psH.tile([128, 64], F32, tag="ps1")
                for m in range(2):
                    nc.tensor.matmul(
                        ps1[:, m * 32:(m + 1) * 32],
                        ws1[:, m * 128:(m + 1) * 128], statict[:],
                        start=True, stop=True)
                s1 = scp.tile([128, 64], BF16, tag="s1")
                nc.scalar.activation(s1[:], ps1[:], AF.Relu)

                # s2
                ps2 = psH.tile([128, 64], F32, tag="ps2")
                for m in range(2):
                    for k in range(2):
                        nc.tensor.matmul(
                            ps2[:, m * 32:(m + 1) * 32],
                            ws2[:, (k * 2 + m) * 128:(k * 2 + m + 1) * 128],
                            s1[:, k * 32:(k + 1) * 32],
                            start=(k == 0), stop=False)
                    nc.tensor.matmul(
                        ps2[:, m * 32:(m + 1) * 32],
                        bs2[:, m * 128:(m + 1) * 128], ones32[:],
                        start=False, stop=True, skip_group_check=True)
                s2 = scp.tile([128, 64], BF16, tag="s2")
                nc.scalar.activation(s2[:], ps2[:], AF.Relu)

                # latent = relu(Wfu @ [s2; acc] + bfu)
                pl = psH.tile([128, 64], F32, tag="pl")
                for m in range(2):
                    for k in range(4):
                        src = s2 if k < 2 else acc
                        kk = k % 2
                        nc.tensor.matmul(
                            pl[:, m * 32:(m + 1) * 32],
                            wfu[:, (k * 2 + m) * 128:(k * 2 + m + 1) * 128],
                            src[:, kk * 32:(kk + 1) * 32],
                            start=(k == 0), stop=False)
                    nc.tensor.matmul(
                        pl[:, m * 32:(m + 1) * 32],
                        bfu[:, m * 128:(m + 1) * 128], ones32[:],
                        start=False, stop=True, skip_group_check=True)
                lat = scp.tile([128, 64], BF16, tag="lat")
                nc.scalar.activation(lat[:], pl[:], AF.Relu)

                # heads layer 0 (per head, M=64, all at partition base 0)
                py1 = psH.tile([64, 192], F32, tag="py1")
                for e in range(6):
                    for k in range(2):
                        nc.tensor.matmul(
                            py1[:, e * 32:(e + 1) * 32],
                            wh0[:, k * 384 + e * 64: k * 384 + (e + 1) * 64],
                            lat[:, k * 32:(k + 1) * 32],
                            start=(k == 0), stop=False)
                    nc.tensor.matmul(
                        py1[:, e * 32:(e + 1) * 32],
                        bh0[:, e * 64:(e + 1) * 64], ones32[:],
                        start=False, stop=True, skip_group_check=True)
                y1 = scp.tile([64, 192], BF16, tag="y1")
                nc.scalar.activation(y1[:], py1[:], AF.Relu)

                # heads layer 1 (per head)
                py2 = psH.tile([64, 192], F32, tag="py2")
                for e in range(6):
                    nc.tensor.matmul(
                        py2[:, e * 32:(e + 1) * 32],
                        wh1[:, e * 64:(e + 1) * 64],
                        y1[:, e * 32:(e + 1) * 32],
                        start=True, stop=False)
                    nc.tensor.matmul(
                        py2[:, e * 32:(e + 1) * 32],
                        bh1[:, e * 64:(e + 1) * 64], ones32[:],
                        start=False, stop=True, skip_group_check=True)
                y2 = scp.tile([64, 192], BF16, tag="y2")
                nc.scalar.activation(y2[:], py2[:], AF.Relu)

                # heads layer 2 -> output
                pout = psH.tile([64, 192], F32, tag="pout")
                offs = [0, 6, 26, 46, 66, 86]
                ns = [6, 20, 20, 20, 20, 20]
                for e in range(6):
                    nc.tensor.matmul(
                        pout[0:ns[e], e * 32:(e + 1) * 32],
                        wh2[:, offs[e]:offs[e] + ns[e]],
                        y2[:, e * 32:(e + 1) * 32],
                        start=True, stop=False)
                    nc.tensor.matmul(
                        pout[0:ns[e], e * 32:(e + 1) * 32],
                        bh2[:, offs[e]:offs[e] + ns[e]], ones32[:],
                        start=False, stop=True, skip_group_check=True)
                seg = scp.tile([20, 192], F32, tag="seg")
                for e in range(6):
                    nc.scalar.activation(
                        seg[0:ns[e], e * 32:(e + 1) * 32],
                        pout[0:ns[e], e * 32:(e + 1) * 32], AF.Copy)
                    nc.sync.dma_start(
                        out=d_out[offs[e]:offs[e] + ns[e], :],
                        in_=seg[0:ns[e], e * 32:(e + 1) * 32])

    nc.compile()
    return nc


# ----------------------------------------------------------------------------
# host wrapper
# ----------------------------------------------------------------------------

def _f32(x):
    return np.asarray(x, np.float32)


def _bf(x):
    return np.ascontiguousarray(np.asarray(x, np.float32).astype(NPBF))


def _kchunk(wT, nk):
    """[nk*128, M] -> [128, nk*M] with col = k*M + m."""
    kdim, m = wT.shape
    assert kdim == nk * 128
    return np.ascontiguousarray(
        wT.reshape(nk, 128, m).transpose(1, 0, 2).reshape(128, nk * m))


def _pack_shared(params):
    c0, c1 = params["cell0"], params["cell1"]
    assert float(np.max(np.abs(_f32(c0["x_mean"])))) == 0.0

    def W(d):
        return _f32(d["w"])

    def bb(d):
        return _f32(d["b"])

    w0cat = np.concatenate([W(c0["Wz"]), W(c0["Wr"]), W(c0["Wh"])], 0)
    b0cat = np.concatenate([bb(c0["Wz"]), bb(c0["Wr"]), bb(c0["Wh"])])
    w1cat = np.concatenate([W(c1["Wz"]), W(c1["Wr"]), W(c1["Wh"])], 0)
    b1cat = np.concatenate([bb(c1["Wz"]), bb(c1["Wr"]), bb(c1["Wh"])])

    sp = params["state"]
    sv = params["surv"]
    st1, st2, fu = params["static1"], params["static2"], params["fusion"]
    heads = [sp] + list(sv)
    wh0cat = np.concatenate([W(h[0]) for h in heads], 0)        # [384,256]
    bh0cat = np.concatenate([bb(h[0]) for h in heads])
    wh1cat = np.concatenate([W(h[1]).T for h in heads], 1)      # [64,384]
    bh1cat = np.concatenate([bb(h[1]) for h in heads])
    wh2cat = np.concatenate([W(h[2]).T for h in heads], 1)      # [64,106]
    bh2cat = np.concatenate([bb(h[2]) for h in heads])

    shared = {
        "w0x": _bf(np.concatenate(
            [w0cat[:, :DT].T, b0cat[None, :]], 0)),             # [65,768]
        "w0h": _bf(_kchunk(w0cat[:, DT:].T, 2)),                # [128,1536]
        "w1x": _bf(_kchunk(w1cat[:, :H].T, 2)),
        "w1h": _bf(_kchunk(w1cat[:, H:].T, 2)),
        "b1": _bf(b1cat[None, :]),
        "wgh0": _bf(W(c0["Wgh"])[:, 0][None, :]),
        "wgx0": _bf(W(c0["Wgx"])[:, 0][None, :]),
        "wgh1": _bf(W(c1["Wgh"])[:, 0][None, :]),
        "bgh0n": np.ascontiguousarray(
            -bb(c0["Wgh"]).reshape(2, 128).T),                  # [128,2]
        "bgx0n": np.ascontiguousarray(-bb(c0["Wgx"])[:, None]),
        "bgh1n": np.ascontiguousarray(
            -bb(c1["Wgh"]).reshape(2, 128).T),
        "ws1": _bf(np.concatenate([W(st1).T, bb(st1)[None, :]], 0)),
        "ws2": _bf(_kchunk(W(st2).T, 2)),
        "bs2": _bf(bb(st2)[None, :]),
        "wfu": _bf(_kchunk(W(fu).T, 4)),
        "bfu": _bf(bb(fu)[None, :]),
        "wh0": _bf(_kchunk(wh0cat.T, 2)),
        "bh0": _bf(bh0cat[None, :]),
        "wh1": _bf(wh1cat),
        "bh1": _bf(bh1cat[None, :]),
        "wh2": _bf(wh2cat),
        "bh2": _bf(bh2cat[None, :]),
    }
    return shared


def _make_in_maps(static, temporal, mask, time_deltas, seq_lens, params):
    shared = _pack_shared(params)
    static = _f32(static)
    temporal = _f32(temporal)
    mask = _f32(mask)
    time_deltas = _f32(time_deltas)
    seq_lens = np.asarray(seq_lens).astype(np.int64)

    in_maps = []
    for i in range(NCORES):
        sl = slice(i * BC, (i + 1) * BC)
        lm1v = (seq_lens[sl] - 1).astype(np.float32)
        lm1 = np.ascontiguousarray(
            np.tile(np.concatenate([lm1v, lm1v])[None, :], (128, 1)))
        m = dict(shared)
        m["temporal_t"] = _bf(temporal[sl].transpose(2, 1, 0))
        m["mask_t"] = _bf(mask[sl].transpose(2, 1, 0))
        m["delta_t"] = _bf(time_deltas[sl].T.reshape(1, -1))
        m["lm1"] = lm1
        m["static_aug"] = _bf(np.concatenate(
            [static[sl].T, np.ones((1, BC), np.float32)], 0))
        in_maps.append(m)
    return in_maps


_NC_CACHE = {}


def _get_nc():
    if "nc" not in _NC_CACHE:
        _NC_CACHE["nc"] = build_nc()
    return _NC_CACHE["nc"]


def _run(in_maps, trace=False):
    nc = _get_nc()
    res = run_bass_kernel_spmd(
        nc, in_maps, core_ids=list(range(NCORES)), trace=trace)
    out = np.concatenate(
        [np.asarray(r["out"], np.float32).T for r in res.results], axis=0)
    return out, res


def kernel(static, temporal, mask, time_deltas, seq_lens, params):
    in_maps = _make_in_maps(static, temporal, mask, time_deltas, seq_lens,
                            params)
    out, _ = _run(in_maps, trace=False)
    return out


def kernel_traced(static, temporal, mask, time_deltas, seq_lens, params):
    in_maps = _make_in_maps(static, temporal, mask, time_deltas, seq_lens,
                            params)
    return _run(in_maps, trace=True)
